# revision 1
# baseline (speedup 1.0000x reference)
"""BrickTube kernel for 8x Trainium2 NeuronCores.

The reference "BrickTube" module applies 80 tiny (2,2,2,2) gate cores to a
[B, 1024] state tensor. Every gate application is linear in x and
INPUT_DIM == BINDIM == OUTPUT_DIM == 1024, so the whole module collapses to

    out = x @ W,   W[i, :] = circuit(e_i)  (1024 x 1024)

W is built exactly on the host in float64 from `cores`, then the device runs a
batch-sharded dense matmul: each of the 8 cores computes y_c^T = W^T @ x_c^T
for its 4096-row shard of x.

Mixed-precision split-K: W's row norms span ~200x. The 768 contraction rows
with the smallest ||W_row|| (holding ~15% of the output energy) are computed
in fp8 e4m3 with MatmulPerfMode.DoubleRow (two 128-row k-subtiles per
instruction at 2x the fp16 MAC rate); the 256 largest rows stay fp16.
Host-measured rel err of this split vs float64: ~1.5e-2 (gate: 2e-2), and the
HW result matches the host simulation to 6 digits.

Scale folding: W8 = e4m3(W_sel8 * S), W16 = fp16(W_sel16 * S) with S a power
of two chosen so W8's absmax sits just under e4m3's 240 max-normal. All
matmuls then accumulate S*y into a single PSUM bank per output block, and the
PSUM->SBUF drain applies the exact 1/S scale for free (scaled copy). Output
is written fp16 (negligible extra error) and upcast on host.

All DRAM tensors are pre-arranged on the host into per-partition-contiguous
[128, ...] layouts so every DMA is a plain 2D block copy with 1-6KB
contiguous elements per partition (full HBM rate, cheap descriptor gen); no
rearranges on the device side.

Device kernel structure (per core):
  - 5 x 512-col warmup matmuls on zeros cover the PE HAM clock ramp while the
    first x8/w8 pieces are in flight (x8 j=0 split on the idle Sync ring,
    weights + remaining x chunks on the Scalar ring).
  - chunk j=0: k-pair-outer fp8 phase then fp16 phase, so the first w8/x8
    pieces cover the first 8 matmuls and w16's later arrival can't stall.
  - chunks j>=1: m-outer "full-finish" — each PSUM bank's 5 matmuls (3 fp8
    DoubleRow + 2 fp16), an immediate scaled drain (DVE/ACT alternating by m
    parity) and a pair-DMA out, so the output flush rides along the chunk's
    compute and the kernel tail only waits on the last 128KB.
"""

import math

import ml_dtypes
import numpy as np

# ---- problem constants (hardcoded per contract) ----
B = 32768
D = 1024
N_CORES = 8
NPC = B // N_CORES  # 4096 batch rows per core

BOND = 2
Q = 10
N_LAYERS = 8
PAIRS1 = [(i, i + 1) for i in range(0, Q, 2)]
PAIRS2 = [(i, (i + 1) % Q) for i in range(1, Q, 2)]
HALF = Q // 2

K8 = 768  # contraction rows computed in fp8 (smallest-norm rows of W)
K16 = D - K8  # rows kept in fp16
T8 = K8 // 128  # 6 fp8 k-subtiles -> 3 DoubleRow pairs
T16 = K16 // 128  # 2 fp16 k-subtiles
JC = NPC // 512  # 8 batch column chunks
MC = D // 128  # 8 output-row chunks


def build_w(cores: np.ndarray) -> np.ndarray:
    """Collapse the 80-gate circuit into W [1024, 1024] (float64),
    with out_row = x_row @ W."""
    c = cores.astype(np.float64)
    s = np.eye(D, dtype=np.float64).reshape((D,) + (BOND,) * Q)
    for layer in range(N_LAYERS):
        base = layer * Q
        for g, (i, j) in enumerate(PAIRS1):
            s = np.tensordot(s, c[base + g], axes=((i + 1, j + 1), (0, 1)))
            s = np.moveaxis(s, (-2, -1), (i + 1, j + 1))
        for g, (i, j) in enumerate(PAIRS2):
            s = np.tensordot(s, c[base + HALF + g], axes=((i + 1, j + 1), (0, 1)))
            s = np.moveaxis(s, (-2, -1), (i + 1, j + 1))
    return s.reshape(D, D)


_NC_CACHE = None


def _build_bass(inv_scale: float):
    """Device program (identical on all 8 cores). Layouts (p = partition):
      x8d  [128, JC*T8*512]  e4m3:  x8d[p, j*3072 + t*512 + n]  = x8[t*128+p, j*512+n]
      x16d [128, JC*T16*512] fp16:  x16d[p, j*1024 + t*512 + n] = x16[t*128+p, j*512+n]
      w8d  [128, T8*1024]    e4m3:  w8d[p, t*1024 + m]  = (W[sel8]*S)[t*128+p, m]
      w16d [128, T16*1024]   fp16:  w16d[p, t*1024 + m] = (W[sel16]*S)[t*128+p, m]
      ytd  [128, JC*MC*512]  fp16:  ytd[p, (j*4+mp)*1024 + h*512 + n]
                                      = y[j*512+n, mp*256 + h*128 + p] (pre 1/S fold)
    """
    global _NC_CACHE
    if _NC_CACHE is not None:
        return _NC_CACHE

    import concourse.bacc as bacc
    import concourse.mybir as mybir
    import concourse.tile as tile

    F8 = mybir.dt.float8e4
    F16 = mybir.dt.float16
    F32 = mybir.dt.float32
    DR = mybir.MatmulPerfMode.DoubleRow

    nc = bacc.Bacc("TRN2")
    x8d = nc.dram_tensor("x8d", [128, JC * T8 * 512], F8, kind="ExternalInput")
    x16d = nc.dram_tensor("x16d", [128, JC * T16 * 512], F16, kind="ExternalInput")
    w8d = nc.dram_tensor("w8d", [128, T8 * D], F8, kind="ExternalInput")
    w16d = nc.dram_tensor("w16d", [128, T16 * D], F16, kind="ExternalInput")
    ytd = nc.dram_tensor("ytd", [128, JC * MC * 512], F16, kind="ExternalOutput")

    with tile.TileContext(nc) as tc:
        with (
            tc.tile_pool(name="xpool", bufs=1) as xpool,
            tc.tile_pool(name="wpool", bufs=1) as wpool,
            tc.tile_pool(name="opool", bufs=2) as opool,
            tc.tile_pool(name="psum", bufs=1, space="PSUM") as ppool,
        ):
            # ---- PE warmup: matmuls on zeros cover the HAM clock ramp
            # while the first x8/w8 DMA pieces are in flight (~2.3us).
            warm = xpool.tile([128, 512], F16, name="warm", tag="warm")
            nc.gpsimd.memset(warm[:], 0)
            wps = ppool.tile([128, 512], F32, name="wps", tag="ps7")
            for _ in range(7):
                nc.tensor.matmul(wps[0:128, :], warm[:, :128], warm[:])

            # ---- weight loads on the Scalar ring: w8's first pair-piece
            # covers phase 1's first 8 matmuls, w16 is only needed at phase 2.
            w8t = wpool.tile([128, T8 * D], F8, name="w8t", tag="w8t")
            nc.scalar.dma_start(w8t[:, : 2 * D], w8d[:, : 2 * D])
            nc.scalar.dma_start(w8t[:, 2 * D :], w8d[:, 2 * D :])
            w16t = wpool.tile([128, T16 * D], F16, name="w16t", tag="w16t")
            nc.scalar.dma_start(w16t[:], w16d[:])

            # ---- x chunk loads. j=0 goes on the otherwise-idle Sync ring in
            # pieces (smallest first) so the first DoubleRow pair's data lands
            # ASAP; j=1..7 stream on the Scalar ring behind the w loads.
            x8j = []
            x16j = []
            for j in range(JC):
                t8 = xpool.tile([128, T8 * 512], F8, name=f"x8j{j}", tag=f"x8{j}")
                src8 = x8d[:, j * T8 * 512 : (j + 1) * T8 * 512]
                if j == 0:
                    nc.sync.dma_start(t8[:, : 2 * 512], src8[:, : 2 * 512])
                    nc.sync.dma_start(t8[:, 2 * 512 :], src8[:, 2 * 512 :])
                else:
                    nc.scalar.dma_start(t8[:], src8)
                x8j.append(t8)
                t16 = xpool.tile([128, T16 * 512], F16, name=f"x16j{j}", tag=f"x16{j}")
                eng = nc.sync if j == 0 else nc.scalar
                eng.dma_start(t16[:], x16d[:, j * T16 * 512 : (j + 1) * T16 * 512])
                x16j.append(t16)

            # ---- main loop over batch chunks
            def dr_mm(psum, m, tp, j, start):
                x8v = x8j[j][:].rearrange("p (t n) -> p t n", n=512)
                w8v = w8t[:].rearrange("p (t m) -> p t m", m=D)
                nc.tensor.matmul(
                    psum[:],
                    w8v[:, 2 * tp : 2 * tp + 2, m * 128 : (m + 1) * 128],
                    x8v[:, 2 * tp : 2 * tp + 2, :],
                    start=start,
                    stop=False,
                    perf_mode=DR,
                )

            def f16_mm(psum, m, t, j):
                nc.tensor.matmul(
                    psum[:],
                    w16t[:, t * D + m * 128 : t * D + (m + 1) * 128],
                    x16j[j][:, t * 512 : (t + 1) * 512],
                    start=False,
                    stop=(t == T16 - 1),
                )

            osb_live = [None]

            def drain(psums, m, j, last_j):
                mp = m // 2
                out_off = (j * (MC // 2) + mp) * 1024
                if m % 2 == 0:
                    osb = opool.tile(
                        [128, 2 * 512], F16, name=f"osb{mp}", tag=f"osb{mp}"
                    )
                    osb_live[0] = osb
                    nc.vector.tensor_scalar_mul(osb[:, :512], psums[m][:], inv_scale)
                    if last_j and mp == MC // 2 - 1:
                        # fire m6's half early; the tail then only waits on m7
                        nc.sync.dma_start(ytd[:, out_off : out_off + 512], osb[:, :512])
                    return
                osb = osb_live[0]
                if last_j and mp == MC // 2 - 1:
                    # final drain split across both engines, each half DMA'd
                    # on its own ring right after its drain — halves the
                    # closing transfer's descriptor chain on each queue
                    nc.vector.tensor_scalar_mul(
                        osb[:, 768:], psums[m][:, 256:], inv_scale
                    )
                    nc.scalar.mul(osb[:, 512:768], psums[m][:, :256], inv_scale)
                    nc.sync.dma_start(ytd[:, out_off + 512 : out_off + 768], osb[:, 512:768])
                    nc.scalar.dma_start(
                        ytd[:, out_off + 768 : out_off + 1024], osb[:, 768:]
                    )
                else:
                    nc.scalar.mul(osb[:, 512:], psums[m][:], inv_scale)
                    nc.sync.dma_start(ytd[:, out_off : out_off + 1024], osb[:])

            for j in range(JC):
                psums = [
                    ppool.tile([128, 512], F32, name=f"ps{m}", tag=f"ps{m}")
                    for m in range(MC)
                ]
                last_j = j == JC - 1
                if j == 0:
                    # k-pair-outer: first w8/x8 pieces cover the first 8 MMs
                    for tp in range(T8 // 2):
                        for m in range(MC):
                            dr_mm(psums[m], m, tp, j, start=(tp == 0))
                    for m in range(MC):
                        for t in range(T16):
                            f16_mm(psums[m], m, t, j)
                        drain(psums, m, j, last_j)
                else:
                    # m-outer full-finish: drains + output DMA chase compute
                    for m in range(MC):
                        for tp in range(T8 // 2):
                            dr_mm(psums[m], m, tp, j, start=(tp == 0))
                        for t in range(T16):
                            f16_mm(psums[m], m, t, j)
                        drain(psums, m, j, last_j)

    nc.compile()
    _NC_CACHE = nc
    return nc


def _prepare(x: np.ndarray, cores: np.ndarray):
    """Host-side: build W, pick the fp8/fp16 row split, quantize and pack
    operands into the per-partition-contiguous device layouts."""
    W = build_w(cores)
    rn = np.sqrt((W * W).sum(axis=1))
    order = np.argsort(rn, kind="stable")
    sel8 = order[:K8]
    sel16 = order[K8:]

    amax8 = float(np.abs(W[sel8]).max())
    amax16 = float(np.abs(W[sel16]).max())
    # keep W8 under e4m3's 240 max-normal and W16*S comfortably inside fp16
    s_pow = min(
        math.floor(math.log2(216.0 / max(amax8, 1e-30))),
        math.floor(math.log2(30000.0 / max(amax16, 1e-30))),
    )
    S = float(2.0**s_pow)

    # w8d[p, t*D + m] = (W[sel8]*S)[t*128+p, m]
    w8d = np.ascontiguousarray(
        (W[sel8] * S)
        .astype(np.float32)
        .astype(ml_dtypes.float8_e4m3)
        .reshape(T8, 128, D)
        .transpose(1, 0, 2)
        .reshape(128, T8 * D)
    )
    w16d = np.ascontiguousarray(
        (W[sel16] * S)
        .astype(np.float32)
        .astype(np.float16)
        .reshape(T16, 128, D)
        .transpose(1, 0, 2)
        .reshape(128, T16 * D)
    )

    # x8 [K8, B] then per-core pack to [128, JC, T8, 512]
    x8_full = x[:, sel8].astype(np.float32).astype(ml_dtypes.float8_e4m3).T
    x16_full = x[:, sel16].astype(np.float32).astype(np.float16).T
    return w8d, w16d, x8_full, x16_full, 1.0 / S


def _pack_x(xf: np.ndarray, c: int, tcount: int):
    """[tcount*128, B] core shard -> [128, JC*tcount*512] device layout."""
    shard = xf[:, c * NPC : (c + 1) * NPC]
    return np.ascontiguousarray(
        shard.reshape(tcount, 128, JC, 512)
        .transpose(1, 2, 0, 3)
        .reshape(128, JC * tcount * 512)
    )


def _run(x: np.ndarray, cores: np.ndarray, trace: bool = False, trace_cores=None):
    from concourse.bass_utils import run_bass_kernel_spmd

    w8d, w16d, x8_full, x16_full, inv_scale = _prepare(x, cores)

    in_maps = []
    for c in range(N_CORES):
        in_maps.append(
            {
                "x8d": _pack_x(x8_full, c, T8),
                "x16d": _pack_x(x16_full, c, T16),
                "w8d": w8d,
                "w16d": w16d,
            }
        )

    nc = _build_bass(inv_scale)
    kwargs = {}
    if trace_cores is not None:
        kwargs["trace_cores"] = trace_cores
    res = run_bass_kernel_spmd(
        nc, in_maps, core_ids=list(range(N_CORES)), trace=trace, **kwargs
    )

    y = np.empty((B, D), dtype=np.float32)
    for c in range(N_CORES):
        # ytd[p, j, mp, h, n] = y[j*512+n, mp*256+h*128+p]
        arr = res.results[c]["ytd"].reshape(128, JC, MC // 2, 2, 512)
        y[c * NPC : (c + 1) * NPC, :] = (
            arr.transpose(1, 4, 2, 3, 0).reshape(NPC, D).astype(np.float32)
        )
    return y, res


def kernel(x: np.ndarray, cores: np.ndarray) -> np.ndarray:
    y, _ = _run(x, cores, trace=False)
    return y



# revision 2
# speedup vs baseline: 1.1041x; 1.1041x over previous
"""BrickTube kernel for 8x Trainium2 NeuronCores — low-rank two-stage version.

The 80-gate circuit collapses to out = x @ W (W 1024x1024, host-built in
float64 from `cores`). Key structural fact: W is a product of 80 random 4x4
gates, so its singular spectrum decays exponentially — rank 128 captures W to
2.7e-9 relative Frobenius error. The device therefore computes the two-stage
factorization

    h = x @ A   (1024 -> 128,  A = U_128 * sigma_128)
    y = h @ B   (128 -> 1024,  B = V_128^T)

which is 4x fewer MACs than the dense matmul the previous version ran.

Numerics / traffic (per core, 4096-row batch shard):
  - x is shipped in three tiers split by W-row-norm (same ordering insight as
    the dense version: small-norm rows carry little output energy):
      512 smallest rows  -> e4m3, consumed by fp8 DoubleRow matmuls (2 pairs)
      next 256 rows      -> e3m4 (4 mantissa bits, 2x more accurate than
                            e4m3; runs at fp16 speed which stage 1 can afford)
      top 256 rows       -> fp16
  - h stays on-chip: PSUM -> bf16 SBUF (per-column scales s_i folded into A's
    columns on the host; 1/s_i folded into B's rows).
  - stage 2 runs in bf16 (wide exponent range kills the underflow that a
    fp16 B would hit: B rows span sigma's 1e9 dynamic range).
  - y leaves the chip as int8: per-output-column scales c_j = 126/(4.5*||W_j||)
    are folded into B's columns, the DVE/ACT PSUM drain casts fp32->int8 with
    round-to-nearest + saturation (verified on HW), and the host divides by
    c_j. Statistical 4.5-sigma clip: y cols are exactly Gaussian, ~25 of 33M
    elements saturate. Host-simulated end-to-end rel err: 1.45e-2 (gate 2e-2).

  Traffic: x 5.24MB + y 4.19MB + weights 0.5MB ~= 9.9MB -> ~27.7us at the
  358 GB/s HBM-per-core limit. PE: 8 chunks x (2 DR + 4 fp16-rate stage-1 MMs
  + 8 bf16 stage-2 MMs) ~= 25.5us. Both ~3x below the dense version.

Device schedule (per core): software-pipelined chunks of 512 batch rows —
PE order s1(0), s1(1), s2(0), s1(2), s2(1), ..., with h drained to bf16 on
DVE in the gap that s1(j+1) covers, stage-2 PSUM drains alternating DVE/ACT,
and int8 outputs pair-DMA'd on the Sync ring while inputs stream on Scalar.
"""

import math

import ml_dtypes
import numpy as np

# ---- problem constants (hardcoded per contract) ----
B = 32768
D = 1024
N_CORES = 8
NPC = B // N_CORES  # 4096 batch rows per core

BOND = 2
Q = 10
N_LAYERS = 8
PAIRS1 = [(i, i + 1) for i in range(0, Q, 2)]
PAIRS2 = [(i, (i + 1) % Q) for i in range(1, Q, 2)]
HALF = Q // 2

R = 128  # factorization rank
N_DR = 512  # x rows in e4m3 (DoubleRow), smallest W-row-norms
N_E3 = 256  # x rows in e3m4
N_16 = D - N_DR - N_E3  # x rows in fp16
T8 = N_DR // 128  # 4 -> 2 DoubleRow pairs
T3 = N_E3 // 128  # 2
T16 = N_16 // 128  # 2
JC = NPC // 512  # 8 batch column chunks
MC = D // 128  # 8 output-row chunks
YCLIP = 4.5  # sigma clip for int8 y quantization


def build_w(cores: np.ndarray) -> np.ndarray:
    """Collapse the 80-gate circuit into W [1024, 1024] (float64),
    with out_row = x_row @ W."""
    c = cores.astype(np.float64)
    s = np.eye(D, dtype=np.float64).reshape((D,) + (BOND,) * Q)
    for layer in range(N_LAYERS):
        base = layer * Q
        for g, (i, j) in enumerate(PAIRS1):
            s = np.tensordot(s, c[base + g], axes=((i + 1, j + 1), (0, 1)))
            s = np.moveaxis(s, (-2, -1), (i + 1, j + 1))
        for g, (i, j) in enumerate(PAIRS2):
            s = np.tensordot(s, c[base + HALF + g], axes=((i + 1, j + 1), (0, 1)))
            s = np.moveaxis(s, (-2, -1), (i + 1, j + 1))
    return s.reshape(D, D)


_NC_CACHE = None


def _build_bass():
    """Device program (identical on all 8 cores). Layouts (p = partition):
      x8d  [128, JC*T8*512]  e4m3: x8d[p, (j*T8+t)*512+n]  = x8q[t*128+p, j*512+n]
      x3d  [128, JC*T3*512]  e3m4: analogous
      x16d [128, JC*T16*512] fp16: analogous
      a8d  [128, T8*128]     e4m3: a8d[p, t*128+m] = A8q[t*128+p, m]
      a3d  [128, T3*128]     e3m4, a16d [128, T16*128] fp16: analogous
      b2d  [128, 1024]       bf16: b2d[p, m] = B2q[p, m]
      ytd  [128, JC*MC*512]  int8: ytd[p, (j*MC+m)*512+n] = q[j*512+n, m*128+p]
    """
    global _NC_CACHE
    if _NC_CACHE is not None:
        return _NC_CACHE

    import concourse.bacc as bacc
    import concourse.mybir as mybir
    import concourse.tile as tile

    F8 = mybir.dt.float8e4
    E3 = mybir.dt.float8e3
    F16 = mybir.dt.float16
    BF16 = mybir.dt.bfloat16
    F32 = mybir.dt.float32
    I8 = mybir.dt.int8
    DR = mybir.MatmulPerfMode.DoubleRow

    nc = bacc.Bacc("TRN2")
    x8d = nc.dram_tensor("x8d", [128, JC * T8 * 512], F8, kind="ExternalInput")
    x3d = nc.dram_tensor("x3d", [128, JC * T3 * 512], E3, kind="ExternalInput")
    x16d = nc.dram_tensor("x16d", [128, JC * T16 * 512], F16, kind="ExternalInput")
    a8d = nc.dram_tensor("a8d", [128, T8 * R], F8, kind="ExternalInput")
    a3d = nc.dram_tensor("a3d", [128, T3 * R], E3, kind="ExternalInput")
    a16d = nc.dram_tensor("a16d", [128, T16 * R], F16, kind="ExternalInput")
    b2d = nc.dram_tensor("b2d", [128, D], BF16, kind="ExternalInput")
    ytd = nc.dram_tensor("ytd", [128, JC * MC * 512], I8, kind="ExternalOutput")

    with tile.TileContext(nc) as tc:
        with (
            tc.tile_pool(name="xpool", bufs=1) as xpool,
            tc.tile_pool(name="wpool", bufs=1) as wpool,
            tc.tile_pool(name="hpool", bufs=1) as hpool,
            tc.tile_pool(name="opool", bufs=1) as opool,
            tc.tile_pool(name="psum", bufs=1, space="PSUM") as ppool,
        ):
            # ---- PE warmup: matmuls on zeros cover the HAM clock ramp
            # while the first weight/x DMA pieces are in flight.
            warm = xpool.tile([128, 512], F16, name="warm", tag="warm")
            nc.gpsimd.memset(warm[:], 0)
            wps = ppool.tile([128, 512], F32, name="wps", tag="psh0")
            for _ in range(7):
                nc.tensor.matmul(wps[0:128, :], warm[:, :128], warm[:])

            # ---- weight loads on the Scalar ring (needed first, tiny)
            a8t = wpool.tile([128, T8 * R], F8, name="a8t", tag="a8t")
            nc.scalar.dma_start(a8t[:], a8d[:])
            a3t = wpool.tile([128, T3 * R], E3, name="a3t", tag="a3t")
            nc.scalar.dma_start(a3t[:], a3d[:])
            a16t = wpool.tile([128, T16 * R], F16, name="a16t", tag="a16t")
            nc.scalar.dma_start(a16t[:], a16d[:])
            b2t = wpool.tile([128, D], BF16, name="b2t", tag="b2t")
            nc.scalar.dma_start(b2t[:], b2d[:])

            # ---- x chunk loads. j=0 in pieces on the Sync ring (first DR
            # pair's data lands ASAP); j>=1 stream on Scalar behind weights.
            x8j, x3j, x16j = [], [], []
            for j in range(JC):
                t8 = xpool.tile([128, T8 * 512], F8, name=f"x8j{j}", tag=f"x8{j}")
                src8 = x8d[:, j * T8 * 512 : (j + 1) * T8 * 512]
                if j == 0:
                    nc.sync.dma_start(t8[:, : 2 * 512], src8[:, : 2 * 512])
                    nc.sync.dma_start(t8[:, 2 * 512 :], src8[:, 2 * 512 :])
                else:
                    nc.scalar.dma_start(t8[:], src8)
                x8j.append(t8)
                t3 = xpool.tile([128, T3 * 512], E3, name=f"x3j{j}", tag=f"x3{j}")
                eng = nc.sync if j == 0 else nc.scalar
                eng.dma_start(t3[:], x3d[:, j * T3 * 512 : (j + 1) * T3 * 512])
                x3j.append(t3)
                t16 = xpool.tile([128, T16 * 512], F16, name=f"x16j{j}", tag=f"x16{j}")
                eng.dma_start(t16[:], x16d[:, j * T16 * 512 : (j + 1) * T16 * 512])
                x16j.append(t16)

            a8v = a8t[:].rearrange("p (t m) -> p t m", m=R)

            def s1(j):
                """Stage 1: psh[j%2] = x_chunk_j @ A (fp8 DR + e3m4 + fp16),
                then drain to bf16 h[j%2] on DVE."""
                psh = ppool.tile([128, 512], F32, name=f"psh{j%2}", tag=f"psh{j%2}")
                x8v = x8j[j][:].rearrange("p (t n) -> p t n", n=512)
                for tp in range(T8 // 2):
                    nc.tensor.matmul(
                        psh[:],
                        a8v[:, 2 * tp : 2 * tp + 2, :],
                        x8v[:, 2 * tp : 2 * tp + 2, :],
                        start=(tp == 0),
                        stop=False,
                        perf_mode=DR,
                    )
                for t in range(T3):
                    nc.tensor.matmul(
                        psh[:],
                        a3t[:, t * R : (t + 1) * R],
                        x3j[j][:, t * 512 : (t + 1) * 512],
                        start=False,
                        stop=False,
                    )
                for t in range(T16):
                    nc.tensor.matmul(
                        psh[:],
                        a16t[:, t * R : (t + 1) * R],
                        x16j[j][:, t * 512 : (t + 1) * 512],
                        start=False,
                        stop=(t == T16 - 1),
                    )
                h = hpool.tile([128, 512], BF16, name=f"h{j%2}", tag=f"h{j%2}")
                nc.vector.tensor_scalar_mul(h[:], psh[:], 1.0)
                return h

            def s2(j, h):
                """Stage 2: 8 bf16 MMs y_m = B2_m^T @ h, drains alternating
                DVE/ACT into int8 pair buffers, pair-DMA out on Sync."""
                for m in range(MC):
                    psy = ppool.tile(
                        [128, 512], F32, name=f"psy{m%4}", tag=f"psy{m%4}"
                    )
                    nc.tensor.matmul(
                        psy[:],
                        b2t[:, m * 128 : (m + 1) * 128],
                        h[:],
                        start=True,
                        stop=True,
                    )
                    mp = m // 2
                    if m % 2 == 0:
                        osb = opool.tile(
                            [128, 2 * 512], I8, name=f"osb{mp%3}", tag=f"osb{mp%3}"
                        )
                        s2.osb = osb
                        nc.vector.tensor_scalar_mul(osb[:, :512], psy[:], 1.0)
                    else:
                        osb = s2.osb
                        nc.scalar.mul(osb[:, 512:], psy[:], 1.0)
                        out_off = (j * MC + m - 1) * 512
                        nc.sync.dma_start(
                            ytd[:, out_off : out_off + 1024], osb[:]
                        )

            h_prev = s1(0)
            for j in range(1, JC + 1):
                h_cur = s1(j) if j < JC else None
                s2(j - 1, h_prev)
                h_prev = h_cur

    nc.compile()
    _NC_CACHE = nc
    return nc


def _prepare(x: np.ndarray, cores: np.ndarray):
    """Host-side: build W, factorize, pick the precision tiers, fold all
    quantization scales into A/B, and pack operands into the
    per-partition-contiguous device layouts."""
    W = build_w(cores)
    U, s, Vt = np.linalg.svd(W)
    A = U[:, :R] * s[:R]  # [D, R]
    Bm = Vt[:R]  # [R, D]

    rn2 = (W * W).sum(axis=1)
    order = np.argsort(rn2, kind="stable")
    sel_dr = order[:N_DR]
    sel_e3 = order[N_DR : N_DR + N_E3]
    sel_16 = order[N_DR + N_E3 :]

    A_dr, A_e3, A_16 = A[sel_dr], A[sel_e3], A[sel_16]
    xf = x.astype(np.float32)
    cx3 = 14.0 / max(float(np.abs(xf[:, sel_e3]).max()), 1e-30)

    # shared per-h-column scale s_i: min over the three format constraints
    si = 216.0 / np.maximum(np.abs(A_dr).max(axis=0), 1e-30)
    si = np.minimum(si, 15.0 * cx3 / np.maximum(np.abs(A_e3).max(axis=0), 1e-30))
    si = np.minimum(si, 30000.0 / np.maximum(np.abs(A_16).max(axis=0), 1e-30))

    E4NP = ml_dtypes.float8_e4m3
    E3NP = ml_dtypes.float8_e3m4

    def to_dram_w(Aq, tcount, np_dt):
        # a?d[p, t*R + m] = Aq[t*128+p, m]
        return np.ascontiguousarray(
            Aq.astype(np.float32)
            .astype(np_dt)
            .reshape(tcount, 128, R)
            .transpose(1, 0, 2)
            .reshape(128, tcount * R)
        )

    a8d = to_dram_w(np.clip(A_dr * si, -240, 240), T8, E4NP)
    a3d = to_dram_w(np.clip(A_e3 * (si / cx3), -15.5, 15.5), T3, E3NP)
    a16d = to_dram_w(A_16 * si, T16, np.float16)

    # stage-2 weights: fold 1/s_i (rows) and y-column scales c_j (columns)
    wcol = np.sqrt((W * W).sum(axis=0))
    cj = 126.0 / (YCLIP * np.maximum(wcol, 1e-30))
    b2d = np.ascontiguousarray(
        (Bm * cj[None, :] / si[:, None]).astype(np.float32).astype(ml_dtypes.bfloat16)
    )

    # x tiers, transposed to [rows, B]
    x8_full = xf[:, sel_dr].astype(E4NP).T
    x3_full = (xf[:, sel_e3] * cx3).astype(E3NP).T
    x16_full = xf[:, sel_16].astype(np.float16).T
    return a8d, a3d, a16d, b2d, x8_full, x3_full, x16_full, cj


def _pack_x(xf: np.ndarray, c: int, tcount: int):
    """[tcount*128, B] core shard -> [128, JC*tcount*512] device layout."""
    shard = xf[:, c * NPC : (c + 1) * NPC]
    return np.ascontiguousarray(
        shard.reshape(tcount, 128, JC, 512)
        .transpose(1, 2, 0, 3)
        .reshape(128, JC * tcount * 512)
    )


def _run(x: np.ndarray, cores: np.ndarray, trace: bool = False, trace_cores=None):
    from concourse.bass_utils import run_bass_kernel_spmd

    a8d, a3d, a16d, b2d, x8_full, x3_full, x16_full, cj = _prepare(x, cores)

    in_maps = []
    for c in range(N_CORES):
        in_maps.append(
            {
                "x8d": _pack_x(x8_full, c, T8),
                "x3d": _pack_x(x3_full, c, T3),
                "x16d": _pack_x(x16_full, c, T16),
                "a8d": a8d,
                "a3d": a3d,
                "a16d": a16d,
                "b2d": b2d,
            }
        )

    nc = _build_bass()
    kwargs = {}
    if trace_cores is not None:
        kwargs["trace_cores"] = trace_cores
    res = run_bass_kernel_spmd(
        nc, in_maps, core_ids=list(range(N_CORES)), trace=trace, **kwargs
    )

    inv_cj = (1.0 / cj).astype(np.float32)
    y = np.empty((B, D), dtype=np.float32)
    for c in range(N_CORES):
        # ytd[p, (j*MC+m)*512+n] = q[j*512+n, m*128+p]
        arr = res.results[c]["ytd"].reshape(128, JC, MC, 512)
        q = arr.transpose(1, 3, 2, 0).reshape(NPC, D)
        y[c * NPC : (c + 1) * NPC, :] = q.astype(np.float32) * inv_cj[None, :]
    return y, res


def kernel(x: np.ndarray, cores: np.ndarray) -> np.ndarray:
    y, _ = _run(x, cores, trace=False)
    return y


# revision 7
# speedup vs baseline: 1.5121x; 1.3695x over previous
"""BrickTube kernel for 8x Trainium2 NeuronCores — low-rank two-stage version.

The 80-gate circuit collapses to out = x @ W (W 1024x1024, host-built in
float64 from `cores`). Key structural fact: W is a product of 80 random 4x4
gates, so its singular spectrum decays exponentially — rank 128 captures W to
2.7e-9 relative Frobenius error. The device therefore computes the two-stage
factorization

    h = x @ A   (1024 -> 128,  A = U_128 * sigma_128)
    y = h @ B   (128 -> 1024,  B = V_128^T)

which is 4x fewer MACs than the dense matmul the previous version ran.

Numerics / traffic (per core, 4096-row batch shard):
  - x is shipped in three tiers split by W-row-norm (same ordering insight as
    the dense version: small-norm rows carry little output energy):
      512 smallest rows  -> e4m3, consumed by fp8 DoubleRow matmuls (2 pairs)
      next 256 rows      -> e3m4 (4 mantissa bits, 2x more accurate than
                            e4m3; runs at fp16 speed which stage 1 can afford)
      top 256 rows       -> fp16
  - h stays on-chip: PSUM -> bf16 SBUF (per-column scales s_i folded into A's
    columns on the host; 1/s_i folded into B's rows).
  - stage 2 runs in bf16 (wide exponent range kills the underflow that a
    fp16 B would hit: B rows span sigma's 1e9 dynamic range).
  - y leaves the chip as int8: per-output-column scales c_j = 126/(4.5*||W_j||)
    are folded into B's columns, the DVE/ACT PSUM drain casts fp32->int8 with
    round-to-nearest + saturation (verified on HW), and the host divides by
    c_j. Statistical 4.5-sigma clip: y cols are exactly Gaussian, ~25 of 33M
    elements saturate. Host-simulated end-to-end rel err: 1.45e-2 (gate 2e-2).

  Traffic: x 5.24MB + y 4.19MB + weights 0.5MB ~= 9.9MB -> ~27.7us at the
  358 GB/s HBM-per-core limit. PE: 8 chunks x (2 DR + 4 fp16-rate stage-1 MMs
  + 8 bf16 stage-2 MMs) ~= 25.5us. Both ~3x below the dense version.

Device schedule (per core): software-pipelined chunks of 512 batch rows —
PE order s1(0), s1(1), s2(0), s1(2), s2(1), ..., with h drained to bf16 on
DVE in the gap that s1(j+1) covers, stage-2 PSUM drains alternating DVE/ACT,
and int8 outputs pair-DMA'd on the Sync ring while inputs stream on Scalar.
"""

import math

import ml_dtypes
import numpy as np

# ---- problem constants (hardcoded per contract) ----
B = 32768
D = 1024
N_CORES = 8
NPC = B // N_CORES  # 4096 batch rows per core

BOND = 2
Q = 10
N_LAYERS = 8
PAIRS1 = [(i, i + 1) for i in range(0, Q, 2)]
PAIRS2 = [(i, (i + 1) % Q) for i in range(1, Q, 2)]
HALF = Q // 2

R = 128  # factorization rank
N_DR = 512  # x rows in e4m3 (DoubleRow), smallest W-row-norms
N_E3 = 256  # x rows in e3m4
N_16 = D - N_DR - N_E3  # x rows in fp16
T8 = N_DR // 128  # 4 -> 2 DoubleRow pairs
T3 = N_E3 // 128  # 2
T16 = N_16 // 128  # 2
JC = NPC // 512  # 8 batch column chunks
MC = D // 128  # 8 output-row chunks
YCLIP = 4.5  # sigma clip for int8 y quantization


def build_w(cores: np.ndarray) -> np.ndarray:
    """Collapse the 80-gate circuit into W [1024, 1024] (float64),
    with out_row = x_row @ W."""
    c = cores.astype(np.float64)
    s = np.eye(D, dtype=np.float64).reshape((D,) + (BOND,) * Q)
    for layer in range(N_LAYERS):
        base = layer * Q
        for g, (i, j) in enumerate(PAIRS1):
            s = np.tensordot(s, c[base + g], axes=((i + 1, j + 1), (0, 1)))
            s = np.moveaxis(s, (-2, -1), (i + 1, j + 1))
        for g, (i, j) in enumerate(PAIRS2):
            s = np.tensordot(s, c[base + HALF + g], axes=((i + 1, j + 1), (0, 1)))
            s = np.moveaxis(s, (-2, -1), (i + 1, j + 1))
    return s.reshape(D, D)


_NC_CACHE = None


def _build_bass():
    """Device program (identical on all 8 cores). DRAM layouts are fused
    byte blocks (p = partition) so each transfer is ONE dma_start — the
    engine-side descriptor generation (~5ns x 128 partition segments) was
    the v2 bottleneck:
      xalld [128, JC*5120] bytes: per chunk [x8 e4m3 2048B | x3 e3m4 1024B
            | x16 fp16 2048B], each tier t-major: tier[p, t*512+n] =
            xq[t*128+p, j*512+n]
      walld [128, 3328] bytes: [a8 512B | a3 256B | a16 512B | b2 2048B],
            a?[p, t*R+m] = Aq[t*128+p, m]; b2[p, m] = B2q[p, m]
      ytd   [128, JC*MC*512] int8: ytd[p, (j*MC+m)*512+n] = q[j*512+n, m*128+p]
    """
    global _NC_CACHE
    if _NC_CACHE is not None:
        return _NC_CACHE

    import concourse.bacc as bacc
    import concourse.mybir as mybir
    import concourse.tile as tile

    F8 = mybir.dt.float8e4
    E3 = mybir.dt.float8e3
    F16 = mybir.dt.float16
    BF16 = mybir.dt.bfloat16
    F32 = mybir.dt.float32
    I8 = mybir.dt.int8
    DR = mybir.MatmulPerfMode.DoubleRow

    # byte offsets of the x tiers inside one fused 5120B/partition chunk
    XB8, XB3, XB16 = T8 * 512, T3 * 512, T16 * 512 * 2
    XB = XB8 + XB3 + XB16  # 5120
    # byte offsets inside the fused 3328B/partition weight block
    WB8, WB3, WB16, WBB = T8 * R, T3 * R, T16 * R * 2, D * 2
    WB = WB8 + WB3 + WB16 + WBB  # 3328

    nc = bacc.Bacc("TRN2")
    xalld = nc.dram_tensor("xalld", [128, JC * XB], I8, kind="ExternalInput")
    walld = nc.dram_tensor("walld", [128, WB], I8, kind="ExternalInput")
    ytd = nc.dram_tensor("ytd", [128, JC * MC * 512], I8, kind="ExternalOutput")

    with tile.TileContext(nc) as tc:
        with (
            tc.tile_pool(name="xpool", bufs=1) as xpool,
            tc.tile_pool(name="wpool", bufs=1) as wpool,
            tc.tile_pool(name="hpool", bufs=1) as hpool,
            tc.tile_pool(name="opool", bufs=1) as opool,
            tc.tile_pool(name="psum", bufs=1, space="PSUM") as ppool,
        ):
            # ---- PE warmup: matmuls on zeros cover the HAM clock ramp
            # while the weight + first x chunk DMAs are in flight.
            warm = xpool.tile([128, 512], F16, name="warm", tag="warm")
            nc.gpsimd.memset(warm[:], 0)
            wps = ppool.tile([128, 512], F32, name="wps", tag="psy3")
            for _ in range(7):
                nc.tensor.matmul(wps[0:128, :], warm[:, :128], warm[:])

            # ---- fused weight block: one DMA on the Scalar ring
            wall = wpool.tile([128, WB], I8, name="wall", tag="wall")
            nc.scalar.dma_start(wall[:], walld[:])
            a8t = wall.bitcast(F8)[:, :WB8]
            a3t = wall.bitcast(E3)[:, WB8 : WB8 + WB3]
            a16t = wall.bitcast(F16)[:, (WB8 + WB3) // 2 : (WB8 + WB3 + WB16) // 2]
            b2t = wall.bitcast(BF16)[:, (WB8 + WB3 + WB16) // 2 : WB // 2]

            # ---- x chunks: one fused DMA per chunk on the Sync ring
            # (j=0 in 4 pieces so the first DR pair's data lands ASAP)
            xall = []
            for j in range(JC):
                t = xpool.tile([128, XB], I8, name=f"xall{j}", tag=f"xall{j}")
                src = xalld[:, j * XB : (j + 1) * XB]
                if j == 0:
                    nc.sync.dma_start(t[:, :1024], src[:, :1024])
                    nc.sync.dma_start(t[:, 1024:2048], src[:, 1024:2048])
                    nc.sync.dma_start(t[:, 2048:3072], src[:, 2048:3072])
                    nc.sync.dma_start(t[:, 3072:], src[:, 3072:])
                else:
                    nc.sync.dma_start(t[:], src)
                xall.append(t)

            a8v = a8t.rearrange("p (t m) -> p t m", m=R)
            drain_engs = [nc.vector, nc.scalar]
            drain_ct = [0]

            def drain(dst, src):
                """PSUM->SBUF drains round-robin between DVE and ACT."""
                eng = drain_engs[drain_ct[0] % 2]
                drain_ct[0] += 1
                if eng is nc.vector:
                    nc.vector.tensor_scalar_mul(dst, src, 1.0)
                else:
                    nc.scalar.mul(dst, src, 1.0)

            def s1(j):
                """Stage 1: psh[j%2] = x_chunk_j @ A (fp8 DR + e3m4 + fp16),
                then drain to bf16 h[j%2]."""
                psh = ppool.tile([128, 512], F32, name=f"psh{j%2}", tag=f"psh{j%2}")
                x8v = (
                    xall[j]
                    .bitcast(F8)[:, :XB8]
                    .rearrange("p (t n) -> p t n", n=512)
                )
                x3v = xall[j].bitcast(E3)[:, XB8 : XB8 + XB3]
                x16v = xall[j].bitcast(F16)[:, (XB8 + XB3) // 2 : XB // 2]
                for tp in range(T8 // 2):
                    nc.tensor.matmul(
                        psh[:],
                        a8v[:, 2 * tp : 2 * tp + 2, :],
                        x8v[:, 2 * tp : 2 * tp + 2, :],
                        start=(tp == 0),
                        stop=False,
                        perf_mode=DR,
                    )
                for t in range(T3):
                    nc.tensor.matmul(
                        psh[:],
                        a3t[:, t * R : (t + 1) * R],
                        x3v[:, t * 512 : (t + 1) * 512],
                        start=False,
                        stop=False,
                    )
                for t in range(T16):
                    nc.tensor.matmul(
                        psh[:],
                        a16t[:, t * R : (t + 1) * R],
                        x16v[:, t * 512 : (t + 1) * 512],
                        start=False,
                        stop=(t == T16 - 1),
                    )
                h = hpool.tile([128, 512], BF16, name=f"h{j%2}", tag=f"h{j%2}")
                drain(h[:], psh[:])
                return h

            def s2(j, h):
                """Stage 2: 8 bf16 MMs y_m = B2_m^T @ h, int8 drains into a
                whole-chunk buffer, one output DMA per chunk on GpSimd."""
                osb = opool.tile(
                    [128, MC * 512], I8, name=f"osb{j%2}", tag=f"osb{j%2}"
                )
                for m in range(MC):
                    psy = ppool.tile(
                        [128, 512], F32, name=f"psy{m%4}", tag=f"psy{m%4}"
                    )
                    nc.tensor.matmul(
                        psy[:],
                        b2t[:, m * 128 : (m + 1) * 128],
                        h[:],
                        start=True,
                        stop=True,
                    )
                    drain(osb[:, m * 512 : (m + 1) * 512], psy[:])
                nc.gpsimd.dma_start(
                    ytd[:, j * MC * 512 : (j + 1) * MC * 512], osb[:]
                )

            h_prev = s1(0)
            for j in range(1, JC + 1):
                h_cur = s1(j) if j < JC else None
                s2(j - 1, h_prev)
                h_prev = h_cur

    nc.compile()
    _NC_CACHE = nc
    return nc


def _prepare(x: np.ndarray, cores: np.ndarray):
    """Host-side: build W, factorize, pick the precision tiers, fold all
    quantization scales into A/B, and pack operands into the
    per-partition-contiguous device layouts."""
    W = build_w(cores)
    U, s, Vt = np.linalg.svd(W)
    A = U[:, :R] * s[:R]  # [D, R]
    Bm = Vt[:R]  # [R, D]

    rn2 = (W * W).sum(axis=1)
    order = np.argsort(rn2, kind="stable")
    sel_dr = order[:N_DR]
    sel_e3 = order[N_DR : N_DR + N_E3]
    sel_16 = order[N_DR + N_E3 :]

    A_dr, A_e3, A_16 = A[sel_dr], A[sel_e3], A[sel_16]
    xf = x.astype(np.float32)
    cx3 = 14.0 / max(float(np.abs(xf[:, sel_e3]).max()), 1e-30)

    # shared per-h-column scale s_i: min over the three format constraints
    si = 216.0 / np.maximum(np.abs(A_dr).max(axis=0), 1e-30)
    si = np.minimum(si, 15.0 * cx3 / np.maximum(np.abs(A_e3).max(axis=0), 1e-30))
    si = np.minimum(si, 30000.0 / np.maximum(np.abs(A_16).max(axis=0), 1e-30))

    E4NP = ml_dtypes.float8_e4m3
    E3NP = ml_dtypes.float8_e3m4

    def to_dram_w(Aq, tcount, np_dt):
        # a?d[p, t*R + m] = Aq[t*128+p, m]
        return np.ascontiguousarray(
            Aq.astype(np.float32)
            .astype(np_dt)
            .reshape(tcount, 128, R)
            .transpose(1, 0, 2)
            .reshape(128, tcount * R)
        )

    a8d = to_dram_w(np.clip(A_dr * si, -240, 240), T8, E4NP)
    a3d = to_dram_w(np.clip(A_e3 * (si / cx3), -15.5, 15.5), T3, E3NP)
    a16d = to_dram_w(A_16 * si, T16, np.float16)

    # stage-2 weights: fold 1/s_i (rows) and y-column scales c_j (columns)
    wcol = np.sqrt((W * W).sum(axis=0))
    cj = 126.0 / (YCLIP * np.maximum(wcol, 1e-30))
    b2d = np.ascontiguousarray(
        (Bm * cj[None, :] / si[:, None]).astype(np.float32).astype(ml_dtypes.bfloat16)
    )

    # fused weight block: [a8 | a3 | a16 | b2] as raw bytes per partition
    walld = np.ascontiguousarray(
        np.concatenate(
            [
                a8d.view(np.uint8),
                a3d.view(np.uint8),
                a16d.view(np.uint8),
                b2d.view(np.uint8),
            ],
            axis=1,
        )
    )

    # x tiers, transposed to [rows, B]
    x8_full = xf[:, sel_dr].astype(E4NP).T
    x3_full = (xf[:, sel_e3] * cx3).astype(E3NP).T
    x16_full = xf[:, sel_16].astype(np.float16).T
    return walld, x8_full, x3_full, x16_full, cj


def _pack_x(xf: np.ndarray, c: int, tcount: int):
    """[tcount*128, B] core shard -> [128, JC, tcount*512*itemsize] bytes."""
    shard = xf[:, c * NPC : (c + 1) * NPC]
    packed = np.ascontiguousarray(
        shard.reshape(tcount, 128, JC, 512).transpose(1, 2, 0, 3)
    )
    return packed.view(np.uint8).reshape(128, JC, -1)


def _run(x: np.ndarray, cores: np.ndarray, trace: bool = False, trace_cores=None):
    from concourse.bass_utils import run_bass_kernel_spmd

    walld, x8_full, x3_full, x16_full, cj = _prepare(x, cores)

    in_maps = []
    for c in range(N_CORES):
        xall = np.concatenate(
            [
                _pack_x(x8_full, c, T8),
                _pack_x(x3_full, c, T3),
                _pack_x(x16_full, c, T16),
            ],
            axis=2,
        ).reshape(128, -1)
        in_maps.append(
            {"xalld": xall.view(np.int8), "walld": walld.view(np.int8)}
        )

    nc = _build_bass()
    kwargs = {}
    if trace_cores is not None:
        kwargs["trace_cores"] = trace_cores
    res = run_bass_kernel_spmd(
        nc, in_maps, core_ids=list(range(N_CORES)), trace=trace, **kwargs
    )

    inv_cj = (1.0 / cj).astype(np.float32)
    y = np.empty((B, D), dtype=np.float32)
    for c in range(N_CORES):
        # ytd[p, (j*MC+m)*512+n] = q[j*512+n, m*128+p]
        arr = res.results[c]["ytd"].reshape(128, JC, MC, 512)
        q = arr.transpose(1, 3, 2, 0).reshape(NPC, D)
        y[c * NPC : (c + 1) * NPC, :] = q.astype(np.float32) * inv_cj[None, :]
    return y, res


def kernel(x: np.ndarray, cores: np.ndarray) -> np.ndarray:
    y, _ = _run(x, cores, trace=False)
    return y
